# revision 1
# baseline (speedup 1.0000x reference)
"""Dense soft-MoE layer for Trainium2, expert-parallel across 8 NeuronCores.

v2: restructured from baseline per HW microbenchmarks:
  - x input pre-chunked on host -> contiguous chunk DMAs
  - gate den/num/gbc matmuls moved between L1/L2 so the PE never waits
    on the DVE softmax chain
  - L1 PSUM groups cycle 4 banks (fast acc8 pattern), L2 its own pool
  - optional l2split: L2 contraction in two 16-tile groups, combined in
    the output stage (avoids the slow 32-long accumulation pattern)

Layers run in fp16 with fp32 PSUM accumulation; layout transposed
(hT[f,t], yT[d,t]) so no on-device transposes are needed.
"""
import sys

sys.path.insert(0, "/opt/trn_rl_repo")

import numpy as np

D = 1024
F = 4096
E = 8
T = 4096
P = 128
TC = 512            # token chunk
NCH = T // TC       # 8 chunks
KD = D // P         # 8 d-tiles (contraction of first matmul)
KF = F // P         # 32 f-tiles (contraction of second matmul)
ND = D // P         # 8 output d-tiles

_cache = {}


def _build(reps: int = 1, loop_n: int = 0, l2split: int = 1, out_bf16: bool = False,
           l2q: int = 1, p1b: int = 4, p2b: int = 2, gb: int = 2):
    import contextlib
    import concourse.mybir as mybir
    import concourse.tile as tile
    from concourse import bacc

    dt = mybir.dt
    AF = mybir.ActivationFunctionType
    ALU = mybir.AluOpType

    nc = bacc.Bacc(None, target_bir_lowering=False, debug=False)

    xq = nc.dram_tensor("xq", [NCH, P, KD, TC], dt.float16, kind="ExternalInput")
    w1e = nc.dram_tensor("w1e", [D, F], dt.float16, kind="ExternalInput")
    w2e = nc.dram_tensor("w2e", [F, D], dt.float16, kind="ExternalInput")
    b1e = nc.dram_tensor("b1e", [F], dt.float32, kind="ExternalInput")
    b2e = nc.dram_tensor("b2e", [D], dt.float32, kind="ExternalInput")
    gw = nc.dram_tensor("gw", [D, E], dt.float16, kind="ExternalInput")
    # gbh holds gate_b / 2: exp computed via tanh so it shares the ACT
    # gelu table: e^x = (1+t)/(1-t), t = tanh(x/2)
    gbh = nc.dram_tensor("gbh", [E, 1], dt.float32, kind="ExternalInput")
    sele = nc.dram_tensor("sele", [E, 1], dt.float16, kind="ExternalInput")
    odt = dt.bfloat16 if out_bf16 else dt.float32
    outT = nc.dram_tensor("outT", [D, T], odt, kind="ExternalOutput")

    with tile.TileContext(nc) as tc:
        with tc.tile_pool(name="weights", bufs=1) as wpool, \
             tc.tile_pool(name="consts", bufs=1) as cpool, \
             tc.tile_pool(name="xin", bufs=2) as xpool, \
             tc.tile_pool(name="hbuf", bufs=1) as hpool, \
             tc.tile_pool(name="psum1", bufs=p1b, space="PSUM") as ppool1, \
             tc.tile_pool(name="psum2", bufs=p2b, space="PSUM") as ppool2, \
             tc.tile_pool(name="gpsum", bufs=gb, space="PSUM") as gpsum, \
             tc.tile_pool(name="upool", bufs=1) as upool, \
             tc.tile_pool(name="small", bufs=4) as spool, \
             tc.tile_pool(name="gate", bufs=2) as gatepool, \
             tc.tile_pool(name="outb", bufs=3) as opool:

            w1_re = w1e.rearrange("(k p) f -> p k f", p=P)
            w1_sb = wpool.tile([P, KD, F], dt.float16)
            for f8 in range(8):
                fs = slice(f8 * (F // 8), (f8 + 1) * (F // 8))
                nc.sync.dma_start(w1_sb[:, :, fs], w1_re[:, :, fs])
            w2_re = w2e.rearrange("(k p) d -> p k d", p=P)
            w2_sb = wpool.tile([P, KF, D], dt.float16)
            for k8 in range(4):
                ks = slice(k8 * (KF // 4), (k8 + 1) * (KF // 4))
                nc.sync.dma_start(w2_sb[:, ks, :], w2_re[:, ks, :])

            b1_sb = cpool.tile([P, KF], dt.float32)
            nc.sync.dma_start(b1_sb[:], b1e.rearrange("(f p) -> p f", p=P))
            b2_sb = cpool.tile([P, ND], dt.float32)
            nc.sync.dma_start(b2_sb[:], b2e.rearrange("(d p) -> p d", p=P))
            gw_sb = cpool.tile([P, KD, E], dt.float16)
            nc.sync.dma_start(gw_sb[:], gw.rearrange("(k p) e -> p k e", p=P))
            gbh_sb = cpool.tile([E, 1], dt.float32)
            nc.sync.dma_start(gbh_sb[:], gbh[:])
            sele_sb = cpool.tile([E, 1], dt.float16)
            nc.sync.dma_start(sele_sb[:], sele[:])
            ones8 = cpool.tile([E, 1], dt.float16)
            nc.any.memset(ones8[:], 1.0)
            ones1 = cpool.tile([1, P], dt.float16)
            nc.any.memset(ones1[:], 1.0)

            KH = KF // l2split         # L2 group length

            loop_cm = tc.For_i(0, loop_n, 1) if loop_n else contextlib.nullcontext()
            with loop_cm:
              for _rep in range(reps):
                for c in range(NCH):
                    x_sb = xpool.tile([P, KD, TC], dt.float16, tag="x")
                    nc.sync.dma_start(x_sb[:], xq[c])

                    # --- gate logits (PE) + exp chain (ACT/DVE, overlaps L1)
                    lg = gpsum.tile([E, TC], dt.float32, tag="gmm")
                    for k in range(KD):
                        nc.tensor.matmul(lg[:], gw_sb[:, k, :], x_sb[:, k, :],
                                         start=(k == 0), stop=(k == KD - 1))
                    expT = spool.tile([E, TC], dt.float16, tag="expT")
                    tt = spool.tile([E, TC], dt.float32, tag="gs")
                    nc.scalar.activation(tt[:], lg[:], AF.Tanh,
                                         bias=gbh_sb[:], scale=0.5)
                    bm = spool.tile([E, TC], dt.float32, tag="gs")
                    nc.vector.tensor_scalar(bm[:], tt[:], -1.0, 1.0,
                                            op0=ALU.mult, op1=ALU.add)
                    rb = spool.tile([E, TC], dt.float32, tag="gs")
                    nc.vector.reciprocal(rb[:], bm[:])
                    ap1 = spool.tile([E, TC], dt.float32, tag="gs")
                    nc.vector.tensor_scalar_add(ap1[:], tt[:], 1.0)
                    nc.vector.tensor_mul(expT[:], ap1[:], rb[:])

                    # --- hT[f, t] = gelu(w1^T x^T + b1) ---
                    hT = hpool.tile([P, KF, TC], dt.float16, tag="hT")
                    for f in range(KF):
                        ph = ppool1.tile([P, TC], dt.float32, tag="mm1")
                        for k in range(KD):
                            nc.tensor.matmul(ph[:], w1_sb[:, k, f * P:(f + 1) * P],
                                             x_sb[:, k, :],
                                             start=(k == 0), stop=(k == KD - 1))
                        nc.scalar.activation(hT[:, f, :], ph[:], AF.Gelu,
                                             bias=b1_sb[:, f:f + 1])

                    # --- gate den/num (PE work is ready; DVE chain done) ---
                    den = gpsum.tile([1, TC], dt.float32, tag="gmm")
                    nc.tensor.matmul(den[:], ones8[:], expT[:], start=True, stop=True)
                    num = gpsum.tile([1, TC], dt.float32, tag="gmm")
                    nc.tensor.matmul(num[:], sele_sb[:], expT[:], start=True, stop=True)
                    rec = spool.tile([1, TC], dt.float32, tag="gs")
                    nc.vector.reciprocal(rec[:], den[:])
                    gcol = spool.tile([1, TC], dt.float16, tag="gcol")
                    nc.vector.tensor_mul(gcol[:], num[:], rec[:])

                    # --- yT[d, t] = w2^T hT ; out = gate * (yT + b2) ---
                    gate_sb = None
                    KQ = KF // l2q          # L2 sub-group length when l2q>1
                    for d in range(ND):
                        pys = []
                        for s in range(max(l2split, l2q)):
                            py = ppool2.tile([P, TC], dt.float32, tag="mm2")
                            KH2 = KQ if l2q > 1 else KH
                            for j in range(KH2):
                                f = s * KH2 + j
                                nc.tensor.matmul(py[:],
                                                 w2_sb[:, f, d * P:(d + 1) * P],
                                                 hT[:, f, :],
                                                 start=(j == 0), stop=(j == KH2 - 1))
                            pys.append(py)
                        if d == 0:
                            # broadcast gate column to all partitions (PE)
                            gbc = gpsum.tile([P, TC], dt.float32, tag="gmm")
                            nc.tensor.matmul(gbc[:], ones1[:], gcol[:],
                                             start=True, stop=True)
                            gate_sb = gatepool.tile([P, TC], dt.float32, tag="gate")
                            nc.vector.tensor_copy(gate_sb[:], gbc[:])
                        if l2q > 1:
                            # drain sub-group banks into one SBUF accumulator
                            # (DVE overlaps the remaining MM groups)
                            u = upool.tile([P, TC], dt.float32, tag="u1")
                            nc.vector.tensor_copy(u[:], pys[0][:])
                            for s in range(1, l2q):
                                nc.vector.tensor_add(u[:], u[:], pys[s][:])
                            ob = opool.tile([P, TC], odt, tag="ob")
                            nc.vector.scalar_tensor_tensor(
                                ob[:], u[:], b2_sb[:, d:d + 1], gate_sb[:],
                                op0=ALU.add, op1=ALU.mult)
                        elif l2split == 1:
                            ob = opool.tile([P, TC], odt, tag="ob")
                            nc.vector.scalar_tensor_tensor(
                                ob[:], pys[0][:], b2_sb[:, d:d + 1], gate_sb[:],
                                op0=ALU.add, op1=ALU.mult)
                        else:
                            # (pyA + b2)*g + pyB*g  (two PSUM srcs in one op
                            # are illegal, so combine via SBUF)
                            s1 = opool.tile([P, TC], dt.float32, tag="ys")
                            nc.vector.scalar_tensor_tensor(
                                s1[:], pys[0][:], b2_sb[:, d:d + 1], gate_sb[:],
                                op0=ALU.add, op1=ALU.mult)
                            s2 = opool.tile([P, TC], dt.float32, tag="ys2")
                            nc.vector.tensor_mul(s2[:], pys[1][:], gate_sb[:])
                            ob = opool.tile([P, TC], odt, tag="ob")
                            nc.vector.tensor_add(ob[:], s1[:], s2[:])
                        tsl = slice(c * TC, (c + 1) * TC)
                        nc.sync.dma_start(outT[d * P:(d + 1) * P, tsl], ob[:])

    nc.compile()
    return nc


def _prep_inmaps(inputs, gate_w, gate_b, w1, b1, w2, b2):
    B, S, Dm = inputs.shape
    x = np.ascontiguousarray(inputs.reshape(-1, Dm))          # [T, D]
    xT16 = np.ascontiguousarray(x.T).astype(np.float16)       # [D, T]
    # pre-chunked layout [NCH, P, KD, TC]: xq[c,p,k,t] = xT[k*P+p, c*TC+t]
    xq = np.ascontiguousarray(
        xT16.reshape(KD, P, NCH, TC).transpose(2, 1, 0, 3))
    gw16 = np.asarray(gate_w, dtype=np.float16)
    gbh32 = np.asarray(gate_b, dtype=np.float32).reshape(E, 1) * 0.5

    in_maps = []
    for e in range(E):
        sel = np.zeros((E, 1), dtype=np.float16)
        sel[e, 0] = 1.0
        in_maps.append({
            "xq": xq,
            "w1e": np.ascontiguousarray(w1[e]).astype(np.float16),
            "w2e": np.ascontiguousarray(w2[e]).astype(np.float16),
            "b1e": np.asarray(b1[e], dtype=np.float32),
            "b2e": np.asarray(b2[e], dtype=np.float32),
            "gw": gw16,
            "gbh": gbh32,
            "sele": sel,
        })
    return in_maps


def kernel(inputs, gate_w, gate_b, w1, b1, w2, b2):
    from concourse.bass_utils import run_bass_kernel_spmd

    if "nc" not in _cache:
        _cache["nc"] = _build()
    nc = _cache["nc"]

    in_maps = _prep_inmaps(inputs, gate_w, gate_b, w1, b1, w2, b2)
    res = run_bass_kernel_spmd(nc, in_maps, core_ids=list(range(E)))

    B, S, Dm = inputs.shape
    acc = res.results[0]["outT"].astype(np.float64)
    for e in range(1, E):
        acc += res.results[e]["outT"]
    out = acc.T.astype(np.float32).reshape(B, S, Dm)
    return out



# revision 2
# speedup vs baseline: 1.0807x; 1.0807x over previous
"""Dense soft-MoE layer for Trainium2, expert-parallel across 8 NeuronCores.

v3: gate path restructured after HW bisection showed the old gate cost
~485us/iter (37% of runtime) against ~15us of theoretical work:
  - gate logits are now a 33rd padded [128,512] psum group per chunk
    (gate_w in cols 0:8 of a zero-padded 128-col weight tile) -- same
    PE config as L1 f-tiles, no sub-128 psum tile for ACT to drain
  - the exp chain (tanh-based, shares the gelu ACT table) runs at full
    128 partitions; rows 8:127 compute harmless junk
  - num = sele @ expT matmul removed: each core's gate_w columns are
    permuted on host so ITS expert is row 0; num is just expT[0:1,:]
  - den (ones8 @ expT[0:8]) and the gate broadcast (ones1 @ gcol)
    remain tiny PE matmuls -- measured free

Layers run in fp16 with fp32 PSUM accumulation; layout transposed
(hT[f,t], yT[d,t]) so no on-device transposes are needed.
"""
import sys

sys.path.insert(0, "/opt/trn_rl_repo")

import numpy as np

D = 1024
F = 4096
E = 8
T = 4096
P = 128
TC = 512            # token chunk
NCH = T // TC       # 8 chunks
KD = D // P         # 8 d-tiles (contraction of first matmul)
KF = F // P         # 32 f-tiles (contraction of second matmul)
ND = D // P         # 8 output d-tiles

_cache = {}


def _build(reps: int = 1, loop_n: int = 0):
    import contextlib
    import concourse.mybir as mybir
    import concourse.tile as tile
    from concourse import bacc

    dt = mybir.dt
    AF = mybir.ActivationFunctionType
    ALU = mybir.AluOpType

    nc = bacc.Bacc(None, target_bir_lowering=False, debug=False)

    xq = nc.dram_tensor("xq", [NCH, P, KD, TC], dt.float16, kind="ExternalInput")
    w1e = nc.dram_tensor("w1e", [D, F], dt.float16, kind="ExternalInput")
    w2e = nc.dram_tensor("w2e", [F, D], dt.float16, kind="ExternalInput")
    b1e = nc.dram_tensor("b1e", [F], dt.float32, kind="ExternalInput")
    b2e = nc.dram_tensor("b2e", [D], dt.float32, kind="ExternalInput")
    # gwp: [D, 128] zero-padded gate weights, expert columns permuted so
    # this core's expert is column 0
    gwp = nc.dram_tensor("gwp", [D, P], dt.float16, kind="ExternalInput")
    # gbhp holds permuted gate_b / 2 padded to [128,1]: exp computed via
    # tanh so it shares the ACT gelu table: e^x = (1+t)/(1-t), t=tanh(x/2)
    gbhp = nc.dram_tensor("gbhp", [P, 1], dt.float32, kind="ExternalInput")
    outT = nc.dram_tensor("outT", [D, T], dt.float32, kind="ExternalOutput")

    with tile.TileContext(nc) as tc:
        with tc.tile_pool(name="weights", bufs=1) as wpool, \
             tc.tile_pool(name="consts", bufs=1) as cpool, \
             tc.tile_pool(name="xin", bufs=2) as xpool, \
             tc.tile_pool(name="hbuf", bufs=1) as hpool, \
             tc.tile_pool(name="psum1", bufs=4, space="PSUM") as ppool1, \
             tc.tile_pool(name="psum2", bufs=2, space="PSUM") as ppool2, \
             tc.tile_pool(name="gpsum", bufs=2, space="PSUM") as gpsum, \
             tc.tile_pool(name="gchain", bufs=1) as gpool, \
             tc.tile_pool(name="small", bufs=2) as spool, \
             tc.tile_pool(name="gate", bufs=2) as gatepool, \
             tc.tile_pool(name="outb", bufs=3) as opool:

            w1_re = w1e.rearrange("(k p) f -> p k f", p=P)
            w1_sb = wpool.tile([P, KD, F], dt.float16)
            for f8 in range(8):
                fs = slice(f8 * (F // 8), (f8 + 1) * (F // 8))
                nc.sync.dma_start(w1_sb[:, :, fs], w1_re[:, :, fs])
            w2_re = w2e.rearrange("(k p) d -> p k d", p=P)
            w2_sb = wpool.tile([P, KF, D], dt.float16)
            for k8 in range(4):
                ks = slice(k8 * (KF // 4), (k8 + 1) * (KF // 4))
                nc.sync.dma_start(w2_sb[:, ks, :], w2_re[:, ks, :])

            b1_sb = cpool.tile([P, KF], dt.float32)
            nc.sync.dma_start(b1_sb[:], b1e.rearrange("(f p) -> p f", p=P))
            b2_sb = cpool.tile([P, ND], dt.float32)
            nc.sync.dma_start(b2_sb[:], b2e.rearrange("(d p) -> p d", p=P))
            gwp_sb = cpool.tile([P, KD, P], dt.float16)
            nc.sync.dma_start(gwp_sb[:], gwp.rearrange("(k p) c -> p k c", p=P))
            gbh_sb = cpool.tile([P, 1], dt.float32)
            nc.sync.dma_start(gbh_sb[:], gbhp[:])
            ones8 = cpool.tile([E, 1], dt.float16)
            nc.any.memset(ones8[:], 1.0)
            ones1 = cpool.tile([1, P], dt.float16)
            nc.any.memset(ones1[:], 1.0)

            loop_cm = tc.For_i(0, loop_n, 1) if loop_n else contextlib.nullcontext()
            with loop_cm:
              for _rep in range(reps):
                for c in range(NCH):
                    x_sb = xpool.tile([P, KD, TC], dt.float16, tag="x")
                    nc.sync.dma_start(x_sb[:], xq[c])

                    # --- gate logits: padded [128,512] group, rows 0:8 real
                    lg = ppool1.tile([P, TC], dt.float32, tag="mm1")
                    for k in range(KD):
                        nc.tensor.matmul(lg[:], gwp_sb[:, k, :], x_sb[:, k, :],
                                         start=(k == 0), stop=(k == KD - 1))
                    # exp chain at full 128 partitions (junk rows harmless):
                    # e^l = (1+t)/(1-t), t = tanh(l/2 + gb/2)
                    tt = gpool.tile([P, TC], dt.float32, tag="tt")
                    nc.scalar.activation(tt[:], lg[:], AF.Tanh,
                                         bias=gbh_sb[:], scale=0.5)
                    bm = gpool.tile([P, TC], dt.float32, tag="bm")
                    nc.vector.tensor_scalar(bm[:], tt[:], -1.0, 1.0,
                                            op0=ALU.mult, op1=ALU.add)
                    rb = gpool.tile([P, TC], dt.float32, tag="rb")
                    nc.vector.reciprocal(rb[:], bm[:])
                    ap1 = gpool.tile([P, TC], dt.float32, tag="ap1")
                    nc.vector.tensor_scalar_add(ap1[:], tt[:], 1.0)
                    expT = gpool.tile([P, TC], dt.float16, tag="expT")
                    nc.vector.tensor_mul(expT[:], ap1[:], rb[:])

                    # --- hT[f, t] = gelu(w1^T x^T + b1) ---
                    hT = hpool.tile([P, KF, TC], dt.float16, tag="hT")
                    for f in range(KF):
                        ph = ppool1.tile([P, TC], dt.float32, tag="mm1")
                        for k in range(KD):
                            nc.tensor.matmul(ph[:], w1_sb[:, k, f * P:(f + 1) * P],
                                             x_sb[:, k, :],
                                             start=(k == 0), stop=(k == KD - 1))
                        nc.scalar.activation(hT[:, f, :], ph[:], AF.Gelu,
                                             bias=b1_sb[:, f:f + 1])

                    # --- gate den; num is just row 0 (expert permuted first)
                    den = gpsum.tile([1, TC], dt.float32, tag="gmm")
                    nc.tensor.matmul(den[:], ones8[:], expT[0:E, :],
                                     start=True, stop=True)
                    rec = spool.tile([1, TC], dt.float32, tag="rec")
                    nc.vector.reciprocal(rec[:], den[:])
                    gcol = spool.tile([1, TC], dt.float16, tag="gcol")
                    nc.vector.tensor_mul(gcol[:], expT[0:1, :], rec[:])

                    # --- yT[d, t] = w2^T hT ; out = gate * (yT + b2) ---
                    gate_sb = None
                    for d in range(ND):
                        py = ppool2.tile([P, TC], dt.float32, tag="mm2")
                        for j in range(KF):
                            nc.tensor.matmul(py[:],
                                             w2_sb[:, j, d * P:(d + 1) * P],
                                             hT[:, j, :],
                                             start=(j == 0), stop=(j == KF - 1))
                        if d == 0:
                            # broadcast gate column to all partitions (PE)
                            gbc = gpsum.tile([P, TC], dt.float32, tag="gmm")
                            nc.tensor.matmul(gbc[:], ones1[:], gcol[:],
                                             start=True, stop=True)
                            gate_sb = gatepool.tile([P, TC], dt.float32, tag="gate")
                            nc.vector.tensor_copy(gate_sb[:], gbc[:])
                        ob = opool.tile([P, TC], dt.float32, tag="ob")
                        nc.vector.scalar_tensor_tensor(
                            ob[:], py[:], b2_sb[:, d:d + 1], gate_sb[:],
                            op0=ALU.add, op1=ALU.mult)
                        tsl = slice(c * TC, (c + 1) * TC)
                        nc.sync.dma_start(outT[d * P:(d + 1) * P, tsl], ob[:])

    nc.compile()
    return nc


def _prep_inmaps(inputs, gate_w, gate_b, w1, b1, w2, b2):
    B, S, Dm = inputs.shape
    x = np.ascontiguousarray(inputs.reshape(-1, Dm))          # [T, D]
    xT16 = np.ascontiguousarray(x.T).astype(np.float16)       # [D, T]
    # pre-chunked layout [NCH, P, KD, TC]: xq[c,p,k,t] = xT[k*P+p, c*TC+t]
    xq = np.ascontiguousarray(
        xT16.reshape(KD, P, NCH, TC).transpose(2, 1, 0, 3))
    gw32 = np.asarray(gate_w, dtype=np.float32)
    gb32 = np.asarray(gate_b, dtype=np.float32)

    in_maps = []
    for e in range(E):
        perm = [e] + [i for i in range(E) if i != e]
        gwp = np.zeros((D, P), dtype=np.float16)
        gwp[:, 0:E] = gw32[:, perm].astype(np.float16)
        gbhp = np.zeros((P, 1), dtype=np.float32)
        gbhp[0:E, 0] = gb32[perm] * 0.5
        in_maps.append({
            "xq": xq,
            "w1e": np.ascontiguousarray(w1[e]).astype(np.float16),
            "w2e": np.ascontiguousarray(w2[e]).astype(np.float16),
            "b1e": np.asarray(b1[e], dtype=np.float32),
            "b2e": np.asarray(b2[e], dtype=np.float32),
            "gwp": gwp,
            "gbhp": gbhp,
        })
    return in_maps


def kernel(inputs, gate_w, gate_b, w1, b1, w2, b2):
    from concourse.bass_utils import run_bass_kernel_spmd

    if "nc" not in _cache:
        _cache["nc"] = _build()
    nc = _cache["nc"]

    in_maps = _prep_inmaps(inputs, gate_w, gate_b, w1, b1, w2, b2)
    res = run_bass_kernel_spmd(nc, in_maps, core_ids=list(range(E)))

    B, S, Dm = inputs.shape
    acc = res.results[0]["outT"].astype(np.float64)
    for e in range(1, E):
        acc += res.results[e]["outT"]
    out = acc.T.astype(np.float32).reshape(B, S, Dm)
    return out
